# revision 16
# baseline (speedup 1.0000x reference)
"""Trainium2 Bass kernel for nn_Discriminator (GAN discriminator with
minibatch discrimination).

Strategy (8 NeuronCores):
  - Batch-shard the conv stack: core r processes samples [r*64, (r+1)*64).
  - Each core computes its f = [hidden, |reco-E|] (577 feats) and
    M = f @ T reshaped (BC=512, 64) in transposed layout (bc on partitions).
  - AllGather M (cast to bf16) -> every core holds M for all 512 samples.
  - Pairwise minibatch-discrimination term: core r computes
    o[j, b] = sum_i exp(-sum_c |M[i,b,c]-M[j,b,c]|) for its own 64 j's:
      per j: DVE tensor_scalar (subtract, abs_max 0) -> |diff| (128bc, 512i)
             PE matmul with 0/1 selector S -> c-group sums (32b, 512i) psum
             ACT Exp(scale=-1) with accum_out -> o column (sum over i).
  - Head (x @ W1.T -> leaky -> @ W2.T -> sigmoid) on own 64 samples.
  - Host concatenates the per-core (1, 64) outputs.

Self-contained: all shapes hardcoded for N=512, A=577, B=32, C=16.
"""

import numpy as np
import ml_dtypes

N = 512          # batch
NC = 8           # cores
NS = N // NC     # samples per core = 64
B = 32           # minibatch-disc out features
C = 16           # kernel dim
BC = B * C       # 512
A = 577          # 576 conv feats + 1 energy diff

F32 = None  # filled by _dt()
BF16 = None

_CACHE = {}


def _build_program(debug_taps=False):
    from contextlib import ExitStack

    import concourse.bass as bass
    import concourse.tile as tile
    from concourse import bacc, mybir

    f32 = mybir.dt.float32
    bf16 = mybir.dt.bfloat16
    u16 = mybir.dt.uint16
    u32 = mybir.dt.uint32
    AF = mybir.ActivationFunctionType
    OP = mybir.AluOpType

    USE_GPS = False
    STRIP_PACK = True
    nc = bacc.Bacc(
        "TRN2", target_bir_lowering=False, debug=False, num_devices=NC
    )

    # ---- I/O ----
    rt = nc.dram_tensor("rt", [81, NS], f32, kind="ExternalInput")       # readout^T
    en = nc.dram_tensor("en", [1, NS], f32, kind="ExternalInput")        # energy
    w1t = nc.dram_tensor("w1t", [4, 4, 32], f32, kind="ExternalInput")   # conv1 lhsT (dx,dy,oc)
    w2t = nc.dram_tensor("w2t", [4, 128, 64], f32, kind="ExternalInput") # conv2 lhsT per dy: rows (dx,ic)
    tsb = nc.dram_tensor("tsb", [64, 9, 4, 128], f32, kind="ExternalInput")  # T blocks
    te = nc.dram_tensor("te", [1, 4, 128], f32, kind="ExternalInput")    # T row 576
    w1p = nc.dram_tensor("w1p", [64, 9, 32], f32, kind="ExternalInput")  # W1 conv-feat blocks
    w1e = nc.dram_tensor("w1e", [1, 32], f32, kind="ExternalInput")      # W1 ediff col
    w1o = nc.dram_tensor("w1o", [32, 32], f32, kind="ExternalInput")     # W1 o-feat block
    w2T = nc.dram_tensor("w2T", [32, 1], f32, kind="ExternalInput")      # W2^T
    b1 = nc.dram_tensor("b1", [32, 1], f32, kind="ExternalInput")
    b2 = nc.dram_tensor("b2", [1, 1], f32, kind="ExternalInput")
    smat = nc.dram_tensor("smat", [128, 4, 32], bf16, kind="ExternalInput")  # selector
    out = nc.dram_tensor("out", [1, NS], f32, kind="ExternalOutput")
    if debug_taps:
        dbg_h1 = nc.dram_tensor("dbg_h1", [32, 36, NS], f32, kind="ExternalOutput")
        dbg_h2 = nc.dram_tensor("dbg_h2", [64, 9, NS], f32, kind="ExternalOutput")
        dbg_ed = nc.dram_tensor("dbg_ed", [1, NS], f32, kind="ExternalOutput")
        dbg_m = nc.dram_tensor("dbg_m", [128, 4, NS], f32, kind="ExternalOutput")
        dbg_o = nc.dram_tensor("dbg_o", [32, NS], f32, kind="ExternalOutput")

    with ExitStack() as ctx:
        tc = ctx.enter_context(tile.TileContext(nc))
        singles = ctx.enter_context(tc.tile_pool(name="singles", bufs=1))
        work = ctx.enter_context(tc.tile_pool(name="work", bufs=4))
        dpool = ctx.enter_context(tc.tile_pool(name="dpool", bufs=18))
        psA = ctx.enter_context(tc.tile_pool(name="psA", bufs=2, space="PSUM"))
        psB = ctx.enter_context(tc.tile_pool(name="psB", bufs=2, space="PSUM"))
        psC = ctx.enter_context(tc.tile_pool(name="psC", bufs=2, space="PSUM"))
        psD = ctx.enter_context(tc.tile_pool(name="psD", bufs=2, space="PSUM"))
        dram = ctx.enter_context(tc.tile_pool(name="dram", bufs=1, space="DRAM"))

        # ---- load everything to SBUF ----
        rt_sb = singles.tile([81, NS], f32)
        nc.sync.dma_start(out=rt_sb[:], in_=rt[:])
        en_sb = singles.tile([1, NS], f32)
        nc.sync.dma_start(out=en_sb[:], in_=en[:])
        w1t_sb = singles.tile([4, 4, 32], f32)
        nc.sync.dma_start(out=w1t_sb[:], in_=w1t[:])
        w2t_sb = singles.tile([128, 4, 64], f32)
        nc.sync.dma_start(
            out=w2t_sb[:],
            in_=bass.AP(tensor=w2t[:].tensor, offset=0,
                        ap=[[64, 128], [128 * 64, 4], [1, 64]]),
        )
        tsb_sb = singles.tile([64, 9, 4, 128], f32)
        nc.sync.dma_start(out=tsb_sb[:], in_=tsb[:])
        te_sb = singles.tile([1, 4, 128], f32)
        nc.sync.dma_start(out=te_sb[:], in_=te[:])
        w1p_sb = singles.tile([64, 9, 32], f32)
        nc.sync.dma_start(out=w1p_sb[:], in_=w1p[:])
        w1e_sb = singles.tile([1, 32], f32)
        nc.sync.dma_start(out=w1e_sb[:], in_=w1e[:])
        w1o_sb = singles.tile([32, 32], f32)
        nc.sync.dma_start(out=w1o_sb[:], in_=w1o[:])
        w2T_sb = singles.tile([32, 1], f32)
        nc.sync.dma_start(out=w2T_sb[:], in_=w2T[:])
        b1_sb = singles.tile([32, 1], f32)
        nc.sync.dma_start(out=b1_sb[:], in_=b1[:])
        b2_sb = singles.tile([1, 1], f32)
        nc.sync.dma_start(out=b2_sb[:], in_=b2[:])
        s_sb = singles.tile([128, 4, 32], bf16)
        nc.sync.dma_start(out=s_sb[:], in_=smat[:])

        # ---- conv1 im2col: one tile per dy (keeps sync-wait fan-in small):
        # r_i2c_dy[dx, (oy,ox), s] = rt[(oy+dy)*9+ox+dx, s]
        r_i2c = []
        for dy in range(4):
            rt_dy = singles.tile([4, 36, NS], f32, tag=f"ri2c{dy}")
            src = bass.AP(
                tensor=rt[:].tensor,
                offset=dy * 9 * NS,
                ap=[[NS, 4], [9 * NS, 6], [NS, 6], [1, NS]],
            )
            dst = rt_dy[:, :, :].rearrange("p (a b) s -> p a b s", a=6)
            nc.sync.dma_start(out=dst, in_=src)
            r_i2c.append(rt_dy)

        # ---- conv1: 4 accumulating K=4 matmuls per chunk -> leaky -> h1
        h1 = singles.tile([32, 6, 6, NS], f32)
        h1_flat = h1[:, :, :, :].rearrange("p a b s -> p (a b s)")
        CH = [(0, 512), (512, 512), (1024, 512), (1536, 512), (2048, 256)]
        for c0, cn in CH:
            ps1 = psC.tile([32, 512], f32, tag="c1")
            for dy in range(4):
                r_flat = r_i2c[dy][:, :, :].rearrange("p a s -> p (a s)")
                nc.tensor.matmul(
                    ps1[:, :cn], w1t_sb[:, dy, :],
                    r_flat[:, c0:c0 + cn],
                    start=(dy == 0), stop=(dy == 3),
                )
            # leaky relu: max(x, 0.2x) (two ops: only one PSUM read allowed)
            lk1 = work.tile([32, 512], f32, tag="lk1")
            nc.vector.tensor_scalar(
                out=lk1[:, :cn], in0=ps1[:, :cn], scalar1=0.2, scalar2=None,
                op0=OP.mult,
            )
            nc.vector.tensor_tensor(
                h1_flat[:, c0:c0 + cn], ps1[:, :cn], lk1[:, :cn], OP.max,
            )

        # ---- conv2 via im2col gather: h2col[dy][(dx,ic), (oy,ox), s]
        #      = h1[ic, oy+dy, ox+dx, s]; then 4 accumulating K=128 matmuls
        #      per psum half.
        h2 = singles.tile([64, 9, NS], f32)
        h2col = []
        for dy in range(4):
            hc = singles.tile([128, 9, NS], f32, tag=f"h2col{dy}")
            for dx in range(4):
                nc.sync.dma_start(
                    out=hc[32 * dx:32 * dx + 32, :, :].rearrange(
                        "p (a b) s -> p a b s", a=3),
                    in_=h1[:, dy:dy + 3, dx:dx + 3, :],
                )
            h2col.append(hc)
        ps2a = psD.tile([64, 5, NS], f32, tag="c2")
        ps2b = psD.tile([64, 4, NS], f32, tag="c2")
        for tgt, lo, hi in ((ps2a, 0, 5), (ps2b, 5, 9)):
            for dy in range(4):
                nc.tensor.matmul(
                    tgt[:, :, :].rearrange("p a s -> p (a s)"),
                    w2t_sb[:, dy, :],
                    h2col[dy][:, lo:hi, :].rearrange("p a s -> p (a s)"),
                    start=(dy == 0), stop=(dy == 3),
                )
        for ps2, lo, hi in ((ps2a, 0, 5), (ps2b, 5, 9)):
            psf_ = ps2[:, :, :].rearrange("p a s -> p (a s)")
            lk2 = work.tile([64, 5 * NS], f32, tag="lk2")
            nn_ = (hi - lo) * NS
            nc.vector.tensor_scalar(
                out=lk2[:, :nn_], in0=psf_, scalar1=0.2, scalar2=None,
                op0=OP.mult,
            )
            nc.vector.tensor_tensor(
                h2[:, lo:hi, :].rearrange("p a s -> p (a s)"),
                psf_, lk2[:, :nn_], OP.max,
            )

        # ---- energy diff: reco = colsum(rt) via ones matmul; ediff = |reco - en|
        ones81 = singles.tile([81, 1], f32)
        nc.vector.memset(ones81[:], 1.0)
        psr = psC.tile([1, NS], f32, tag="c1")
        nc.tensor.matmul(psr[:], ones81[:], rt_sb[:], start=True, stop=True)
        ediff = singles.tile([1, NS], f32)
        tmp_e = work.tile([1, NS], f32, tag="tmp_e")
        nc.vector.tensor_tensor(
            out=tmp_e[:], in0=psr[:], in1=en_sb[:], op=OP.subtract
        )
        nc.vector.tensor_scalar(
            out=ediff[:].bitcast(u32), in0=tmp_e[:].bitcast(u32),
            scalar1=0x7FFFFFFF, scalar2=None, op0=OP.bitwise_and,
        )

        # ---- M = f @ T in transposed layout: m_own[(m,u), s] (128, 4, 64) bf16
        m_own = singles.tile([128, 4, NS], bf16)
        for m in range(4):
            psm = psD.tile([128, NS], f32, tag="c2")
            for p9 in range(9):
                nc.tensor.matmul(
                    psm[:], tsb_sb[:, p9, m, :], h2[:, p9, :],
                    start=(p9 == 0), stop=False,
                )
            nc.tensor.matmul(
                psm[:], te_sb[:, m, :], ediff[:], start=False, stop=True,
            )
            nc.vector.tensor_copy(out=m_own[:, m, :], in_=psm[:])
        # f32 copy of the bf16-rounded values (tensor_scalar needs f32 scalar;
        # also keeps the i==j diagonal exactly zero)
        m_own_r32 = singles.tile([128, 4, NS], f32)
        nc.vector.tensor_copy(out=m_own_r32[:], in_=m_own[:])
        neg_m = singles.tile([128, 4, NS], f32)
        nc.vector.tensor_scalar(
            out=neg_m[:], in0=m_own_r32[:], scalar1=-1.0, scalar2=None,
            op0=OP.mult,
        )

        # ---- AllGather M across 8 cores ----
        cc_in = dram.tile([128, 4, NS], bf16)
        cc_out = dram.tile([NC, 128, 4, NS], bf16)
        nc.sync.dma_start(out=cc_in[:], in_=m_own[:])
        nc.gpsimd.collective_compute(
            "AllGather",
            mybir.AluOpType.bypass,
            replica_groups=[list(range(NC))],
            ins=[cc_in[:]],
            outs=[cc_out[:]],
        )
        # m_full free layout (t, i): contiguous (128, 512) slice per t so the
        # DVE absdiff reads hit fast perf modes; one DMA so consumers wait on
        # a single queue semaphore
        m_full = singles.tile([128, 4, NC * NS], bf16)
        src_g = bass.AP(
            tensor=cc_out[:].tensor,
            offset=0,
            ap=[[4 * NS, 128], [NS, 4], [128 * 4 * NS, NC], [1, NS]],
        )
        nc.sync.dma_start(
            out=m_full[:, :, :].rearrange("p t (r s) -> p t r s", r=NC),
            in_=src_g,
        )

        # ---- pairwise: blocks of 4 j's packed into one (128, 512) psum ----
        # engine split per (j, t): t=0 -> ACT (|x+bias| one op); t=1,2 -> DVE
        # (sub then uint16 bit-and); t=3 -> DVE (GPS offload toggled off)
        o_stack = singles.tile([128, NS // 4], f32)
        for jb in range(NS // 4):
            ds = {}
            for q in range(4):
                jl = jb * 4 + q
                for t in range(4):
                    d = dpool.tile([128, 512], bf16, tag="d")
                    if t == 0:
                        nc.scalar.activation(
                            out=d[:], in_=m_full[:, t, :],
                            func=AF.Abs,
                            bias=neg_m[:, t, jl:jl + 1], scale=1.0,
                        )
                    else:
                        tsub = work.tile([128, 512], bf16, tag="tsub")
                        nc.vector.tensor_scalar(
                            out=tsub[:], in0=m_full[:, t, :],
                            scalar1=m_own_r32[:, t, jl:jl + 1],
                            scalar2=None, op0=OP.subtract,
                        )
                        eng = nc.gpsimd if (t == 3 and USE_GPS) else nc.vector
                        eng.tensor_scalar(
                            out=d[:].bitcast(u16), in0=tsub[:].bitcast(u16),
                            scalar1=0x7FFF, scalar2=None,
                            op0=OP.bitwise_and,
                        )
                    ds[(q, t)] = d
            psL = psA.tile([128, 512], f32, tag="psL")
            if STRIP_PACK:
                for t in range(4):
                    for q in range(4):
                        nc.tensor.matmul(
                            psL[32 * q:32 * q + 32, :], s_sb[:, t, :],
                            ds[(q, t)][:],
                            start=(t == 0), stop=(t == 3),
                            tile_position=(0, 32 * q),
                        )
            else:
                for q in range(4):
                    for t in range(4):
                        nc.tensor.matmul(
                            psL[32 * q:32 * q + 32, :], s_sb[:, t, :],
                            ds[(q, t)][:],
                            start=(t == 0), stop=(t == 3),
                            tile_position=(0, 32 * q),
                        )
            e_ps = psB.tile([128, 512], f32, tag="e_ps")
            nc.scalar.activation(
                out=e_ps[:], in_=psL[:], func=AF.Exp, scale=-1.0,
                accum_out=o_stack[:, jb:jb + 1],
            )
        # reorder o_stack[(q,b), jb] -> o_t[b, 4*jb+q] via 4 strided DMAs
        o_t = singles.tile([32, NS], f32)
        for q in range(4):
            dst = bass.AP(
                tensor=o_t[:].tensor, offset=o_t[:].offset + q,
                ap=[o_t[:].ap[0], [4, NS // 4]],
            )
            nc.sync.dma_start(out=dst, in_=o_stack[32 * q:32 * q + 32, :])

        # ---- head ----
        psh = psC.tile([32, NS], f32, tag="c1")
        for p9 in range(9):
            nc.tensor.matmul(
                psh[:], w1p_sb[:, p9, :], h2[:, p9, :],
                start=(p9 == 0), stop=False,
            )
        nc.tensor.matmul(psh[:], w1e_sb[:], ediff[:], start=False, stop=False)
        nc.tensor.matmul(psh[:], w1o_sb[:], o_t[:], start=False, stop=True)
        t1 = work.tile([32, NS], f32, tag="t1")
        nc.vector.tensor_scalar(
            out=t1[:], in0=psh[:], scalar1=b1_sb[:, 0:1], scalar2=None,
            op0=OP.add,
        )
        x1 = work.tile([32, NS], f32, tag="x1")
        nc.vector.scalar_tensor_tensor(
            out=x1[:], in0=t1[:], scalar=0.2, in1=t1[:],
            op0=OP.mult, op1=OP.max,
        )
        psf = psC.tile([1, NS], f32, tag="c1")
        nc.tensor.matmul(psf[:], w2T_sb[:], x1[:], start=True, stop=True)
        outT = work.tile([1, NS], f32, tag="outT")
        nc.scalar.activation(
            out=outT[:], in_=psf[:], func=AF.Sigmoid, bias=b2_sb[:, 0:1],
            scale=1.0,
        )
        nc.sync.dma_start(out=out[:], in_=outT[:])
        if debug_taps:
            nc.sync.dma_start(out=dbg_h1[:], in_=h1[:, :, :, :].rearrange("p a b s -> p (a b) s"))
            nc.sync.dma_start(out=dbg_h2[:], in_=h2[:])
            nc.sync.dma_start(out=dbg_ed[:], in_=ediff[:])
            nc.sync.dma_start(out=dbg_m[:], in_=m_own_r32[:])
            nc.sync.dma_start(out=dbg_o[:], in_=o_t[:])

    nc.compile()
    return nc


def _prep_weights(inputs):
    """Host-side weight packing (shared across cores)."""
    conv1_w = np.asarray(inputs["conv1_w"], np.float32)   # (32,1,4,4)
    conv2_w = np.asarray(inputs["conv2_w"], np.float32)   # (64,32,4,4)
    T = np.asarray(inputs["T"], np.float32)               # (577, 512)
    W1 = np.asarray(inputs["W1"], np.float32)             # (32, 609)
    b1 = np.asarray(inputs["b1"], np.float32)             # (32,)
    W2 = np.asarray(inputs["W2"], np.float32)             # (1, 32)
    b2 = np.asarray(inputs["b2"], np.float32)             # (1,)

    w1t = np.ascontiguousarray(conv1_w.reshape(32, 4, 4).transpose(2, 1, 0))  # (dx,dy,oc)
    w2t = np.ascontiguousarray(conv2_w.transpose(2, 3, 1, 0).reshape(4, 128, 64))  # (dy, (dx,ic), oc)
    tsb = np.ascontiguousarray(T[:576].reshape(64, 9, 4, 128))
    te = np.ascontiguousarray(T[576].reshape(1, 4, 128))
    w1p = np.ascontiguousarray(W1[:, :576].T.reshape(64, 9, 32))
    w1e = np.ascontiguousarray(W1[:, 576:577].T)                     # (1,32)
    w1o = np.ascontiguousarray(W1[:, 577:].T)                        # (32,32)
    w2T = np.ascontiguousarray(W2.T)                                 # (32,1)
    b1r = b1.reshape(32, 1).copy()
    b2r = b2.reshape(1, 1).copy()
    # selector: smat[u, t, b] = 1 if b == 8*t + u//16
    u = np.arange(128)
    smat = np.zeros((128, 4, 32), np.float32)
    for t in range(4):
        smat[u, t, 8 * t + u // 16] = 1.0
    smat = smat.astype(ml_dtypes.bfloat16)
    return dict(w1t=w1t, w2t=w2t, tsb=tsb, te=te, w1p=w1p, w1e=w1e,
                w1o=w1o, w2T=w2T, b1=b1r, b2=b2r, smat=smat)


def kernel(**inputs) -> np.ndarray:
    from concourse.bass_utils import run_bass_kernel_spmd

    if "nc" not in _CACHE:
        _CACHE["nc"] = _build_program()
    nc = _CACHE["nc"]

    readout = np.asarray(inputs["readout"], np.float32).reshape(N, 81)
    energy = np.asarray(inputs["energy"], np.float32)
    weights = _prep_weights(inputs)

    in_maps = []
    for r in range(NC):
        sl = slice(r * NS, (r + 1) * NS)
        m = dict(weights)
        m["rt"] = np.ascontiguousarray(readout[sl].T)        # (81, 64)
        m["en"] = np.ascontiguousarray(energy[sl].reshape(1, NS))
        in_maps.append(m)

    res = run_bass_kernel_spmd(nc, in_maps, core_ids=list(range(NC)))
    outs = [res.results[r]["out"].reshape(NS) for r in range(NC)]
    return np.concatenate(outs).astype(np.float32)
